# revision 10
# baseline (speedup 1.0000x reference)
"""SSD ConfidenceLoss on 8 TRN2 NeuronCores (Bass/Tile).

Math
----
loss[b,d,c] = -gts * log_softmax(predicts);  per box:
  lse      = log(sum_c exp(p_c))          (|p| < ~6, no max-sub needed)
  box_loss = lse * sum_c(g_c) - sum_c(g_c * p_c)     (= full CE at the box)
  neg_val  = g_last * (lse - p_last)  >= 0 always (lse > p_c strictly)
pos_loss = sum(box_loss * pos);  N = sum(pos)
neg_loss = sum of top-neg_num of where(pos, -inf, neg_val),
           neg_num = min(3N, total-N).
Since every neg_val >= 0 and masked entries are -inf (never reach rank
neg_num <= total-N), the top-k sum equals the sum of ALL nonzero masked
values whenever nnz = count(masked > 0) <= neg_num.  The kernel computes
(N, pos_loss, S=sum(masked), nnz) on device; the host uses S directly
when nnz <= neg_num (exact), else falls back to an exact np.partition
over the masked values (also produced by the device).

Device layout (per core, SPMD, no collectives)
----------------------------------------------
8732*8 = 69,856 boxes/core, zero-padded to 69,888 = 128 x 546 (zero
boxes contribute exactly 0 to every statistic).  T tiles of
[128 partitions, W boxes * 21 classes], W*T = 546.  predicts/gts DMA
with SWDGE f32->bf16 cast (HBM traffic stays f32).  ACT: exp, log.
PE: the three per-box class-sums (exp, gts, gts*p) via 21 accumulated
identity matmuls into PSUM (contraction-free accumulate).  DVE: the
p*g elementwise mul plus small per-box ops with fused accum_out
partial reductions into a [128, 4T] stats tile.
"""

import sys

import numpy as np
import ml_dtypes

for _p in ("/opt/trn_rl_repo",):
    if _p not in sys.path:
        sys.path.append(_p)

B, D, C = 64, 8732, 21
NEG_FACTOR = 3
N_CORES = 8
P = 128  # SBUF partitions

BOXES_PER_CORE = B * D // N_CORES          # 69,856
BOXES_PAD = ((BOXES_PER_CORE + P - 1) // P) * P  # 69,888 = 128*546
COLS = BOXES_PAD // P                      # 546 boxes per partition
W = 273                                    # boxes per partition per tile
T = COLS // W                              # 2 tiles
assert W * T == COLS
FREE = W * C                               # 3822 elements per partition per tile

_CACHE = {}


def _build(onehot=True):
    """onehot=True: gts rows are exactly one-hot (host-verified) -> gsum==1,
    skip the gts class-sum pass.  onehot=False: fully general program."""
    key = ("nc", onehot)
    if key in _CACHE:
        return _CACHE[key]

    import concourse.mybir as mybir
    import concourse.tile as tile
    from concourse import bacc

    f32 = mybir.dt.float32
    bf16 = mybir.dt.bfloat16
    u8 = mybir.dt.uint8

    nc = bacc.Bacc("TRN2", target_bir_lowering=False, debug=False,
                   num_devices=N_CORES)

    pred = nc.dram_tensor("predicts", [BOXES_PAD * C], f32, kind="ExternalInput").ap()
    gts = nc.dram_tensor("gts", [BOXES_PAD * C], f32, kind="ExternalInput").ap()
    pos = nc.dram_tensor("pos", [BOXES_PAD], u8, kind="ExternalInput").ap()
    ident = nc.dram_tensor("ident", [P, P], bf16, kind="ExternalInput").ap()
    stats = nc.dram_tensor("stats", [P, 4 * T], f32, kind="ExternalOutput").ap()
    negvals = nc.dram_tensor("negvals", [BOXES_PAD], f32, kind="ExternalOutput").ap()

    Exp = mybir.ActivationFunctionType.Exp
    Ln = mybir.ActivationFunctionType.Ln
    mult = mybir.AluOpType.mult
    add = mybir.AluOpType.add
    is_gt = mybir.AluOpType.is_gt
    X = mybir.AxisListType.X

    with tile.TileContext(nc) as tc:
        with (
            tc.tile_pool(name="big", bufs=3) as big,
            tc.tile_pool(name="small", bufs=4) as small,
            tc.tile_pool(name="psum", bufs=4, space="PSUM") as psum,
            tc.tile_pool(name="const", bufs=1) as const,
        ):
            id_t = const.tile([P, P], bf16)
            nc.sync.dma_start(id_t[:], ident[:])
            stats_t = const.tile([P, 4 * T], f32)

            def seg_sum_pe(dst_ps, src3):
                """dst_ps[p, w] = sum_c src3[p, w, c] via PE accumulate."""
                for c in range(C):
                    nc.tensor.matmul(dst_ps[:], id_t[:], src3[:, :, c],
                                     start=(c == 0), stop=(c == C - 1))

            for t in range(T):
                eb = t * P * FREE
                p_bf = big.tile([P, FREE], bf16, tag="p")
                nc.gpsimd.dma_start(
                    p_bf[:], pred[eb:eb + P * FREE].rearrange("(p f) -> p f", f=FREE))
                g_bf = big.tile([P, FREE], bf16, tag="g")
                nc.gpsimd.dma_start(
                    g_bf[:], gts[eb:eb + P * FREE].rearrange("(p f) -> p f", f=FREE))
                posf = small.tile([P, W], f32, tag="posf")
                pb = t * P * W
                nc.gpsimd.dma_start(
                    posf[:], pos[pb:pb + P * W].rearrange("(p w) -> p w", w=W))

                p3 = p_bf[:].rearrange("p (w c) -> p w c", c=C)
                g3 = g_bf[:].rearrange("p (w c) -> p w c", c=C)

                # exp (natural layout); class-sum on PE (strided rhs)
                e_bf = big.tile([P, FREE], bf16, tag="e")
                nc.scalar.activation(e_bf[:], p_bf[:], Exp)
                s_ps = psum.tile([P, W], f32, tag="s")
                seg_sum_pe(s_ps, e_bf[:].rearrange("p (w c) -> p w c", c=C))

                # p*g product (DVE 2x natural); its class-sum on DVE
                pg_bf = big.tile([P, FREE], bf16, tag="pg")
                nc.vector.tensor_mul(pg_bf[:], p_bf[:], g_bf[:])
                gp_sb = small.tile([P, W], f32, tag="gp")
                nc.vector.tensor_reduce(
                    gp_sb[:], pg_bf[:].rearrange("p (w c) -> p w c", c=C),
                    axis=X, op=add)

                lse = small.tile([P, W], f32, tag="lse")
                nc.scalar.activation(lse[:], s_ps[:], Ln)

                # N partial: sum_w posf
                nc.vector.tensor_reduce(stats_t[:, 4 * t:4 * t + 1], posf[:],
                                        axis=X, op=add)

                # box_loss = lse * gsum - gp   (gsum == 1 in one-hot mode)
                if onehot:
                    bl = small.tile([P, W], f32, tag="bl")
                    nc.vector.tensor_sub(bl[:], lse[:], gp_sb[:])
                else:
                    gs_ps = psum.tile([P, W], f32, tag="gs")
                    seg_sum_pe(gs_ps, g3)
                    t1 = small.tile([P, W], f32, tag="t1")
                    nc.vector.tensor_mul(t1[:], lse[:], gs_ps[:])
                    bl = small.tile([P, W], f32, tag="bl")
                    nc.vector.tensor_sub(bl[:], t1[:], gp_sb[:])

                # pos_loss partial: sum_w box_loss * posf
                prod = small.tile([P, W], f32, tag="prod")
                nc.vector.scalar_tensor_tensor(
                    prod[:], bl[:], 1.0, posf[:], op0=mult, op1=mult,
                    accum_out=stats_t[:, 4 * t + 1:4 * t + 2])

                # neg_val = g_last * (lse - p_last); masked = neg_val * (1-posf)
                p3 = p_bf[:].rearrange("p (w c) -> p w c", c=C)
                g3 = g_bf[:].rearrange("p (w c) -> p w c", c=C)
                pl = small.tile([P, W], f32, tag="pl")
                nc.vector.tensor_copy(pl[:], p3[:, :, C - 1])
                gl = small.tile([P, W], f32, tag="gl")
                nc.vector.tensor_copy(gl[:], g3[:, :, C - 1])
                u = small.tile([P, W], f32, tag="u")
                nc.vector.tensor_sub(u[:], lse[:], pl[:])
                nraw = small.tile([P, W], f32, tag="nraw")
                nc.vector.tensor_mul(nraw[:], u[:], gl[:])
                notf = small.tile([P, W], f32, tag="notf")
                nc.vector.tensor_scalar(notf[:], posf[:], -1.0, 1.0,
                                        op0=mult, op1=add)
                masked = small.tile([P, W], f32, tag="masked")
                nc.vector.scalar_tensor_tensor(
                    masked[:], nraw[:], 1.0, notf[:], op0=mult, op1=mult,
                    accum_out=stats_t[:, 4 * t + 2:4 * t + 3])

                # nnz partial: count masked > 0
                ind = small.tile([P, W], f32, tag="ind")
                nc.vector.tensor_scalar(ind[:], masked[:], 0.0, None, op0=is_gt,
                                        op1=add,
                                        accum_out=stats_t[:, 4 * t + 3:4 * t + 4])

                nc.sync.dma_start(
                    negvals[pb:pb + P * W].rearrange("(p w) -> p w", w=W),
                    masked[:])

            nc.sync.dma_start(stats[:], stats_t[:])

    nc.compile()
    _CACHE[key] = nc
    return nc


def _gts_is_onehot(gts):
    """Exact check: every row of gts is one-hot (values in {0,1}, row sum 1)."""
    g = np.asarray(gts)
    if ((g != 0.0) & (g != 1.0)).any():
        return False
    return bool((g.sum(-1) == 1.0).all())


def _shard_inputs(predicts, gts, pos_indicator):
    """Full (64,8732,21)/(64,8732) inputs -> 8 per-core padded flat maps."""
    pred_flat = np.ascontiguousarray(predicts, dtype=np.float32).reshape(-1)
    gts_flat = np.ascontiguousarray(gts, dtype=np.float32).reshape(-1)
    pos_flat = np.asarray(pos_indicator).reshape(-1).view(np.uint8)
    ident = np.eye(P, dtype=ml_dtypes.bfloat16)

    in_maps = []
    for i in range(N_CORES):
        pb = i * BOXES_PER_CORE
        pe_pad = np.zeros(BOXES_PAD * C, dtype=np.float32)
        pe_pad[:BOXES_PER_CORE * C] = pred_flat[pb * C:(pb + BOXES_PER_CORE) * C]
        ge_pad = np.zeros(BOXES_PAD * C, dtype=np.float32)
        ge_pad[:BOXES_PER_CORE * C] = gts_flat[pb * C:(pb + BOXES_PER_CORE) * C]
        po_pad = np.zeros(BOXES_PAD, dtype=np.uint8)
        po_pad[:BOXES_PER_CORE] = pos_flat[pb:pb + BOXES_PER_CORE]
        in_maps.append({
            "predicts": pe_pad,
            "gts": ge_pad,
            "pos": po_pad,
            "ident": ident,
        })
    return in_maps


def _combine(results):
    """Host combine of per-core [128, 4T] stats (+ exact fallback)."""
    N = 0.0
    pos_loss = 0.0
    S = 0.0
    nnz = 0.0
    for r in results:
        st = r["stats"].astype(np.float64)
        N += st[:, 0::4].sum()
        pos_loss += st[:, 1::4].sum()
        S += st[:, 2::4].sum()
        nnz += st[:, 3::4].sum()

    total = B * D
    neg_num = min(NEG_FACTOR * N, total - N)
    if nnz <= neg_num:
        neg_loss = S
    else:
        # exact fallback: top-neg_num of masked vals (all selected are > 0,
        # so zeros from masking/padding can never displace a real value)
        vals = np.concatenate([r["negvals"].astype(np.float64) for r in results])
        k = int(round(neg_num))
        neg_loss = np.partition(vals, len(vals) - k)[len(vals) - k:].sum()

    return np.float32((pos_loss + neg_loss) / N)


def kernel(predicts, gts, pos_indicator):
    from concourse.bass_utils import run_bass_kernel_spmd

    nc = _build(onehot=_gts_is_onehot(gts))
    in_maps = _shard_inputs(predicts, gts, pos_indicator)
    res = run_bass_kernel_spmd(nc, in_maps, core_ids=list(range(N_CORES)))
    return _combine(res.results)


# revision 11
# speedup vs baseline: 1.0827x; 1.0827x over previous
"""SSD ConfidenceLoss on 8 TRN2 NeuronCores (Bass/Tile).

Math
----
loss[b,d,c] = -gts * log_softmax(predicts);  per box:
  lse      = log(sum_c exp(p_c))          (|p| < ~6, no max-sub needed)
  box_loss = lse * sum_c(g_c) - sum_c(g_c * p_c)     (= full CE at the box)
  neg_val  = g_last * (lse - p_last)  >= 0 always (lse > p_c strictly)
pos_loss = sum(box_loss * pos);  N = sum(pos)
neg_loss = sum of top-neg_num of where(pos, -inf, neg_val),
           neg_num = min(3N, total-N).
Since every neg_val >= 0 and masked entries are -inf (never reach rank
neg_num <= total-N), the top-k sum equals the sum of ALL nonzero masked
values whenever nnz = count(masked > 0) <= neg_num.  The kernel computes
(N, pos_loss, S=sum(masked), nnz) on device; the host uses S directly
when nnz <= neg_num (exact), else falls back to an exact np.partition
over the masked values (also produced by the device).

Device layout (per core, SPMD, no collectives)
----------------------------------------------
8732*8 = 69,856 boxes/core, zero-padded to 69,888 = 128 x 546 (zero
boxes contribute exactly 0 to every statistic).  T tiles of
[128 partitions, W boxes * 21 classes], W*T = 546.  predicts/gts DMA
with SWDGE f32->bf16 cast (HBM traffic stays f32).  ACT: exp, log.
PE: the three per-box class-sums (exp, gts, gts*p) via 21 accumulated
identity matmuls into PSUM (contraction-free accumulate).  DVE: the
p*g elementwise mul plus small per-box ops with fused accum_out
partial reductions into a [128, 4T] stats tile.
"""

import sys

import numpy as np
import ml_dtypes

for _p in ("/opt/trn_rl_repo",):
    if _p not in sys.path:
        sys.path.append(_p)

B, D, C = 64, 8732, 21
NEG_FACTOR = 3
N_CORES = 8
P = 128  # SBUF partitions

BOXES_PER_CORE = B * D // N_CORES          # 69,856
BOXES_PAD = ((BOXES_PER_CORE + P - 1) // P) * P  # 69,888 = 128*546
COLS = BOXES_PAD // P                      # 546 boxes per partition
W = 273                                    # boxes per partition per tile
T = COLS // W                              # 2 tiles
assert W * T == COLS
FREE = W * C                               # 3822 elements per partition per tile

_CACHE = {}


def _build(onehot=True):
    """onehot=True: gts rows are exactly one-hot (host-verified) -> gsum==1,
    skip the gts class-sum pass.  onehot=False: fully general program."""
    key = ("nc", onehot)
    if key in _CACHE:
        return _CACHE[key]

    import concourse.mybir as mybir
    import concourse.tile as tile
    from concourse import bacc

    f32 = mybir.dt.float32
    bf16 = mybir.dt.bfloat16
    u8 = mybir.dt.uint8

    nc = bacc.Bacc("TRN2", target_bir_lowering=False, debug=False,
                   num_devices=N_CORES)

    pred = nc.dram_tensor("predicts", [BOXES_PAD * C], f32, kind="ExternalInput").ap()
    gts = nc.dram_tensor("gts", [BOXES_PAD * C], f32, kind="ExternalInput").ap()
    pos = nc.dram_tensor("pos", [BOXES_PAD], u8, kind="ExternalInput").ap()
    ident = nc.dram_tensor("ident", [P, P], bf16, kind="ExternalInput").ap()
    stats = nc.dram_tensor("stats", [P, 4 * T], f32, kind="ExternalOutput").ap()
    negvals = nc.dram_tensor("negvals", [BOXES_PAD], f32, kind="ExternalOutput").ap()

    Exp = mybir.ActivationFunctionType.Exp
    Ln = mybir.ActivationFunctionType.Ln
    mult = mybir.AluOpType.mult
    add = mybir.AluOpType.add
    is_gt = mybir.AluOpType.is_gt
    X = mybir.AxisListType.X

    with tile.TileContext(nc) as tc:
        with (
            tc.tile_pool(name="big", bufs=2) as big,
            tc.tile_pool(name="small", bufs=2) as small,
            tc.tile_pool(name="psum", bufs=2, space="PSUM") as psum,
            tc.tile_pool(name="const", bufs=1) as const,
        ):
            id_t = const.tile([P, P], bf16)
            nc.sync.dma_start(id_t[:], ident[:])
            stats_t = const.tile([P, 4 * T], f32)

            def seg_sum_pe(dst_ps, src3):
                """dst_ps[p, w] = sum_c src3[p, w, c] via PE accumulate."""
                for c in range(C):
                    nc.tensor.matmul(dst_ps[:], id_t[:], src3[:, :, c],
                                     start=(c == 0), stop=(c == C - 1))

            for t in range(T):
                eb = t * P * FREE
                p_bf = big.tile([P, FREE], bf16, tag="p")
                nc.gpsimd.dma_start(
                    p_bf[:], pred[eb:eb + P * FREE].rearrange("(p f) -> p f", f=FREE))
                g_bf = big.tile([P, FREE], bf16, tag="g")
                nc.gpsimd.dma_start(
                    g_bf[:], gts[eb:eb + P * FREE].rearrange("(p f) -> p f", f=FREE))
                posf = small.tile([P, W], f32, tag="posf")
                pb = t * P * W
                nc.gpsimd.dma_start(
                    posf[:], pos[pb:pb + P * W].rearrange("(p w) -> p w", w=W))

                p3 = p_bf[:].rearrange("p (w c) -> p w c", c=C)
                g3 = g_bf[:].rearrange("p (w c) -> p w c", c=C)

                # exp (natural layout); class-sum on PE (strided rhs)
                e_bf = big.tile([P, FREE], bf16, tag="e")
                nc.scalar.activation(e_bf[:], p_bf[:], Exp)
                s_ps = psum.tile([P, W], f32, tag="s")
                seg_sum_pe(s_ps, e_bf[:].rearrange("p (w c) -> p w c", c=C))

                # p*g product (DVE 2x natural); its class-sum on DVE
                pg_bf = big.tile([P, FREE], bf16, tag="pg")
                nc.vector.tensor_mul(pg_bf[:], p_bf[:], g_bf[:])
                gp_sb = small.tile([P, W], f32, tag="gp")
                nc.vector.tensor_reduce(
                    gp_sb[:], pg_bf[:].rearrange("p (w c) -> p w c", c=C),
                    axis=X, op=add)

                lse = small.tile([P, W], f32, tag="lse")
                nc.scalar.activation(lse[:], s_ps[:], Ln)

                # N partial: sum_w posf
                nc.vector.tensor_reduce(stats_t[:, 4 * t:4 * t + 1], posf[:],
                                        axis=X, op=add)

                # box_loss = lse * gsum - gp   (gsum == 1 in one-hot mode)
                if onehot:
                    bl = small.tile([P, W], f32, tag="bl")
                    nc.vector.tensor_sub(bl[:], lse[:], gp_sb[:])
                else:
                    gs_ps = psum.tile([P, W], f32, tag="gs")
                    seg_sum_pe(gs_ps, g3)
                    t1 = small.tile([P, W], f32, tag="t1")
                    nc.vector.tensor_mul(t1[:], lse[:], gs_ps[:])
                    bl = small.tile([P, W], f32, tag="bl")
                    nc.vector.tensor_sub(bl[:], t1[:], gp_sb[:])

                # pos_loss partial: sum_w box_loss * posf
                prod = small.tile([P, W], f32, tag="prod")
                nc.vector.scalar_tensor_tensor(
                    prod[:], bl[:], 1.0, posf[:], op0=mult, op1=mult,
                    accum_out=stats_t[:, 4 * t + 1:4 * t + 2])

                # neg_val = g_last * (lse - p_last); masked = neg_val * (1-posf)
                p3 = p_bf[:].rearrange("p (w c) -> p w c", c=C)
                g3 = g_bf[:].rearrange("p (w c) -> p w c", c=C)
                pl = small.tile([P, W], f32, tag="pl")
                nc.vector.tensor_copy(pl[:], p3[:, :, C - 1])
                gl = small.tile([P, W], f32, tag="gl")
                nc.vector.tensor_copy(gl[:], g3[:, :, C - 1])
                u = small.tile([P, W], f32, tag="u")
                nc.vector.tensor_sub(u[:], lse[:], pl[:])
                nraw = small.tile([P, W], f32, tag="nraw")
                nc.vector.tensor_mul(nraw[:], u[:], gl[:])
                notf = small.tile([P, W], f32, tag="notf")
                nc.vector.tensor_scalar(notf[:], posf[:], -1.0, 1.0,
                                        op0=mult, op1=add)
                masked = small.tile([P, W], f32, tag="masked")
                nc.vector.scalar_tensor_tensor(
                    masked[:], nraw[:], 1.0, notf[:], op0=mult, op1=mult,
                    accum_out=stats_t[:, 4 * t + 2:4 * t + 3])

                # nnz partial: count masked > 0
                ind = small.tile([P, W], f32, tag="ind")
                nc.vector.tensor_scalar(ind[:], masked[:], 0.0, None, op0=is_gt,
                                        op1=add,
                                        accum_out=stats_t[:, 4 * t + 3:4 * t + 4])

                nc.sync.dma_start(
                    negvals[pb:pb + P * W].rearrange("(p w) -> p w", w=W),
                    masked[:])

            nc.sync.dma_start(stats[:], stats_t[:])

    nc.compile()
    _CACHE[key] = nc
    return nc


def _gts_is_onehot(gts):
    """Exact check: every row of gts is one-hot (values in {0,1}, row sum 1)."""
    g = np.asarray(gts)
    if ((g != 0.0) & (g != 1.0)).any():
        return False
    return bool((g.sum(-1) == 1.0).all())


def _shard_inputs(predicts, gts, pos_indicator):
    """Full (64,8732,21)/(64,8732) inputs -> 8 per-core padded flat maps."""
    pred_flat = np.ascontiguousarray(predicts, dtype=np.float32).reshape(-1)
    gts_flat = np.ascontiguousarray(gts, dtype=np.float32).reshape(-1)
    pos_flat = np.asarray(pos_indicator).reshape(-1).view(np.uint8)
    ident = np.eye(P, dtype=ml_dtypes.bfloat16)

    in_maps = []
    for i in range(N_CORES):
        pb = i * BOXES_PER_CORE
        pe_pad = np.zeros(BOXES_PAD * C, dtype=np.float32)
        pe_pad[:BOXES_PER_CORE * C] = pred_flat[pb * C:(pb + BOXES_PER_CORE) * C]
        ge_pad = np.zeros(BOXES_PAD * C, dtype=np.float32)
        ge_pad[:BOXES_PER_CORE * C] = gts_flat[pb * C:(pb + BOXES_PER_CORE) * C]
        po_pad = np.zeros(BOXES_PAD, dtype=np.uint8)
        po_pad[:BOXES_PER_CORE] = pos_flat[pb:pb + BOXES_PER_CORE]
        in_maps.append({
            "predicts": pe_pad,
            "gts": ge_pad,
            "pos": po_pad,
            "ident": ident,
        })
    return in_maps


def _combine(results):
    """Host combine of per-core [128, 4T] stats (+ exact fallback)."""
    N = 0.0
    pos_loss = 0.0
    S = 0.0
    nnz = 0.0
    for r in results:
        st = r["stats"].astype(np.float64)
        N += st[:, 0::4].sum()
        pos_loss += st[:, 1::4].sum()
        S += st[:, 2::4].sum()
        nnz += st[:, 3::4].sum()

    total = B * D
    neg_num = min(NEG_FACTOR * N, total - N)
    if nnz <= neg_num:
        neg_loss = S
    else:
        # exact fallback: top-neg_num of masked vals (all selected are > 0,
        # so zeros from masking/padding can never displace a real value)
        vals = np.concatenate([r["negvals"].astype(np.float64) for r in results])
        k = int(round(neg_num))
        neg_loss = np.partition(vals, len(vals) - k)[len(vals) - k:].sum()

    return np.float32((pos_loss + neg_loss) / N)


def kernel(predicts, gts, pos_indicator):
    from concourse.bass_utils import run_bass_kernel_spmd

    nc = _build(onehot=_gts_is_onehot(gts))
    in_maps = _shard_inputs(predicts, gts, pos_indicator)
    res = run_bass_kernel_spmd(nc, in_maps, core_ids=list(range(N_CORES)))
    return _combine(res.results)
